# revision 20
# baseline (speedup 1.0000x reference)
"""Multi-head causal self-attention on 8 Trainium2 NeuronCores.

Problem: B=2, T=2048, C=1024, H=16 heads, D=64 head_dim, fp32.
  qkv = x @ Wqkv; causal softmax attention per head; y = attn_out @ Wout.

Sharding (2-way data parallel on batch x 4-way tensor parallel on heads):
  core c -> batch b = c // 4, head group g = c % 4 (heads 4g..4g+3).
  Each core computes its batch's QKV projection for its 4 heads, attention
  for those heads, and a partial out-projection (its 256 rows of Wout).
  The host sums the 4 partials per batch (the TP all-reduce).

Per-core kernel layout. All matmul operands live at SBUF partition base 0
(the PE rejects base-64 operands on this toolchain). "T"-suffixed tensors
are transposed, i.e. [feature, token] with feature on partitions:
  - x is PE-transposed once into xT [chan, tok]; 4 transposes share one
    PSUM bank and are evacuated with a single [128, 512] copy.
  - K^T head-pair blocks via matmul(lhsT=Wk_blk, rhs=xT) -> [col, tok].
  - Q^T per head in zero-padded [128, T] tiles qZ[h]: rows (h%2)*64..+64
    hold Q_h^T * 1/sqrt(D), the other 64 rows are zero, so a K=128
    contraction against the K^T pair block yields S^T for head h alone.
  - V in natural [tok, d] layout via matmul(lhsT=xT_blk, rhs=Wv), stored
    interleaved as [V_h | 1] per head (ones column -> denominator row).
  - S^T[k, q] = matmul(lhsT=K^T pair-block k-cols, rhs=qZ[h] q-tile), two
    k-blocks fused per [128, 1024] PSUM pair; one fused exp on ACT
    (no max subtraction: scores ~ N(0,1), fp32 exp is safe); diagonal
    blocks get a multiplicative 0/1 causal mask (split DVE / GpSimd).
  - O^T/denom = matmul(lhsT=[V_h|1], rhs=P^T) accumulated per head-pair in
    one [65, 2, 512] PSUM tile: rows 0..63 = O^T, row 64 = denominator.
  - normalize per head-pair: one reciprocal at partition 64 (lane-
    aligned), one SBUF->SBUF DMA down to partition 0 (DMA can shift
    partitions), one gpsimd partition_broadcast, one DVE mul -> yT.
  - partial out-proj: per token block accumulate 4 K=64 matmuls
    (lhsT=yT head slice, rhs=Wout rows) into a [128, 1024] PSUM pair;
    outputs are staged and DMA'd to HBM two token blocks at a time.

Matmuls run in fp32r (full PE rate; fp32 matmul is 4x slower). Set
MM_DT = mybir.dt.float32 below for the full-precision fallback.

build(repeat=N) unrolls the whole kernel N times back-to-back (used by
test.py to measure per-iteration device time without dispatch overhead).
"""

import math
from contextlib import ExitStack

import numpy as np

import concourse.bass as bass
import concourse.mybir as mybir
import concourse.tile as tile
from concourse import bacc
from concourse.bass_utils import run_bass_kernel_spmd
from concourse.masks import make_identity

B, T, C = 2, 2048, 1024
H, D = 16, 64
NCORES = 8
TPG = 4            # tensor-parallel group size (cores per batch)
HG = H // TPG      # heads per core = 4
HCOLS = HG * D     # 256 qkv columns per core
CB = C // 128      # 8 chan blocks
TB = T // 128      # 16 token blocks
QT = T // 512      # 4 q tiles
SCALE = 1.0 / math.sqrt(D)

F32 = mybir.dt.float32
MM_DT = mybir.dt.float32r   # matmul operand dtype (float32r = full PE rate)

_CACHE = {}


def _mm(nc, out, lhsT, rhs, **kw):
    nc.tensor.matmul(out, lhsT.bitcast(MM_DT), rhs.bitcast(MM_DT), **kw)


def _build_rep(nc, tc, ctx, tensors, masks_state):
    xb, wq, wk, wv, wo, yp = tensors
    Exp = mybir.ActivationFunctionType.Exp
    Copy = mybir.ActivationFunctionType.Copy

    with ExitStack() as pctx:
        # ---------- pools that live through the whole repetition ----------
        pp = pctx.enter_context(tc.tile_pool(name="pp", bufs=1))
        kT = pp.tile([128, 2 * T], MM_DT, tag="kT")
        qZ = [pp.tile([128, T], MM_DT, tag=f"qZ{h}", name=f"qZ{h}")
              for h in range(HG)]
        v1 = pp.tile([128, TB, HG, 65], MM_DT, tag="v1")

        with ExitStack() as phase12:
            sb1 = phase12.enter_context(tc.tile_pool(name="sb1", bufs=1))
            xT = sb1.tile([128, CB, T], MM_DT, tag="xT")       # 64KB/part
            wq_sb = sb1.tile([128, CB, HCOLS], MM_DT, tag="wq")
            wk_sb = sb1.tile([128, CB, HCOLS], MM_DT, tag="wk")
            wv_sb = sb1.tile([128, CB, HCOLS], MM_DT, tag="wv")
            ident = sb1.tile([128, 128], F32, tag="ident")
            make_identity(nc, ident[:])
            nc.sync.dma_start(wq_sb[:], wq.rearrange("(cb p) n -> p cb n", p=128).bitcast(MM_DT))
            nc.sync.dma_start(wk_sb[:], wk.rearrange("(cb p) n -> p cb n", p=128).bitcast(MM_DT))
            nc.sync.dma_start(wv_sb[:], wv.rearrange("(cb p) n -> p cb n", p=128).bitcast(MM_DT))
            # constants: ones columns of v1, zero halves of qZ
            nc.gpsimd.memset(v1[:, :, :, 64].bitcast(F32), 1.0)
            for h in range(HG):
                dead = slice(64, 128) if h % 2 == 0 else slice(0, 64)
                nc.gpsimd.memset(qZ[h][dead, :].bitcast(F32), 0.0)

            # PSUM budget 8 banks: tp(2) + vps(2) + qps(2) + kps(2).
            xn_pool = phase12.enter_context(tc.tile_pool(name="xn", bufs=3))
            tp_ps = phase12.enter_context(
                tc.tile_pool(name="tp_ps", bufs=2, space="PSUM"))
            v_ps = phase12.enter_context(
                tc.tile_pool(name="v_ps", bufs=2, space="PSUM"))
            qk_ps = phase12.enter_context(
                tc.tile_pool(name="qk_ps", bufs=2, space="PSUM"))

            # ---- phase 1: load x (2 tok blocks per DMA), transpose to xT,
            # ---- compute V per tok block
            for tb2 in range(TB // 2):
                xn = xn_pool.tile([128, 2, C], F32, tag="xn")
                nc.sync.dma_start(
                    xn[:], xb[tb2 * 256:(tb2 + 1) * 256, :]
                    .rearrange("(two p) c -> p two c", p=128))
                for half in range(2):
                    tb = 2 * tb2 + half
                    for cb4 in range(CB // 4):   # 4 transposes per PSUM bank
                        tp = tp_ps.tile([128, 512], F32, tag="tp")
                        for j in range(4):
                            cb = cb4 * 4 + j
                            nc.tensor.transpose(
                                tp[:, j * 128:(j + 1) * 128],
                                xn[:, half, cb * 128:(cb + 1) * 128],
                                ident[:])
                        # one evac for 4 transposed blocks: xT free-dim
                        # layout is [cb, tok], so this is a strided write
                        nc.vector.tensor_copy(
                            xT[:, cb4 * 4:(cb4 + 1) * 4,
                               tb * 128:(tb + 1) * 128],
                            tp[:].rearrange("p (cb t) -> p cb t", t=128))
                    vps = v_ps.tile([128, HCOLS], F32, tag="vps")
                    for cb in range(CB):
                        _mm(nc, vps[:], xT[:, cb, tb * 128:(tb + 1) * 128],
                            wv_sb[:, cb, :],
                            start=(cb == 0), stop=(cb == CB - 1))
                    nc.vector.tensor_copy(
                        v1[:, tb, :, 0:64],
                        vps[:].rearrange("p (h d) -> p h d", d=64))

            # ---- phase 2: K^T pair blocks + zero-padded scaled Q^T ----
            for m in range(2):
                for qt in range(QT):
                    qps = qk_ps.tile([128, 512], F32, tag="qps")
                    kps = qk_ps.tile([128, 512], F32, tag="kps")
                    for cb in range(CB):
                        _mm(nc, qps[:], wq_sb[:, cb, m * 128:(m + 1) * 128],
                            xT[:, cb, qt * 512:(qt + 1) * 512],
                            start=(cb == 0), stop=(cb == CB - 1))
                    for cb in range(CB):
                        _mm(nc, kps[:], wk_sb[:, cb, m * 128:(m + 1) * 128],
                            xT[:, cb, qt * 512:(qt + 1) * 512],
                            start=(cb == 0), stop=(cb == CB - 1))
                    tsl = slice(qt * 512, (qt + 1) * 512)
                    nc.scalar.activation(qZ[2 * m][0:64, tsl], qps[0:64, :],
                                         Copy, scale=SCALE)
                    nc.scalar.activation(qZ[2 * m + 1][64:128, tsl],
                                         qps[64:128, :], Copy, scale=SCALE)
                    nc.vector.tensor_copy(
                        kT[:, m * T + qt * 512: m * T + (qt + 1) * 512],
                        kps[:])

        # ---------- attention + out-projection ----------
        ap = pctx.enter_context(tc.tile_pool(name="ap", bufs=1))
        # Wout rows per head-PAIR at base 0: [128, pair, 1024] so the
        # out-projection contracts K=128 (heads 2hp and 2hp+1 together).
        wo_sb = ap.tile([128, 2, C], MM_DT, tag="wo")
        nc.sync.dma_start(wo_sb[:], wo.rearrange("(pb p) n -> p pb n", p=128).bitcast(MM_DT))
        # yTp[hp]: rows 0..63 = head 2hp, rows 64..127 = head 2hp+1
        yTp = ap.tile([128, 2, T], MM_DT, tag="yTp")
        # single triangular mask [128, 128]: 1 where qcol >= krow else 0.
        # On a diagonal k-block ki (j = ki - 4*qt), cols < 128j are fully
        # masked (skipped in the matmuls), cols in [128j, 128j+128) follow
        # this triangle, cols >= 128j+128 are fully valid.
        tri = ap.tile([128, 128], F32, tag="tri")
        nc.gpsimd.memset(tri[:], 1.0)
        nc.gpsimd.affine_select(
            out=tri[:], in_=tri[:], compare_op=mybir.AluOpType.is_ge,
            fill=0.0, base=0, pattern=[[1, 128]], channel_multiplier=-1)

        # Attention PSUM budget: st2(2x2) + ot2(1x2) + ops2(1x2) = 8 banks.
        st_ps = pctx.enter_context(tc.tile_pool(name="st_ps", bufs=2, space="PSUM"))
        ot_ps = pctx.enter_context(tc.tile_pool(name="ot_ps", bufs=1, space="PSUM"))
        op_ps = pctx.enter_context(tc.tile_pool(name="op_ps", bufs=1, space="PSUM"))
        pt_pool = pctx.enter_context(tc.tile_pool(name="pt", bufs=5))
        nrm_pool = pctx.enter_context(tc.tile_pool(name="nrm", bufs=2))
        out_pool = pctx.enter_context(tc.tile_pool(name="out", bufs=3))

        for qt in range(QT):
            nki = 4 * (qt + 1)
            for hp in range(2):          # head pairs (0,1) and (2,3)
                ot2 = ot_ps.tile([65, 2, 512], F32, tag="ot2")
                for ki2 in range(0, nki, 2):
                    for hh in range(2):
                        h = 2 * hp + hh
                        m = h // 2
                        st2 = st_ps.tile([128, 2, 512], F32, tag="st2")
                        for j in range(2):
                            ki = ki2 + j
                            jd = ki - 4 * qt
                            # skip fully-masked columns of diagonal blocks
                            # (keep qt==0 full so PSUM slots are always
                            # written over their full width at least once;
                            # stale cols feed exp and must stay finite)
                            off = 128 * jd if (jd > 0 and qt > 0) else 0
                            _mm(nc, st2[:, j, off:512],
                                kT[:, m * T + ki * 128: m * T + (ki + 1) * 128],
                                qZ[h][:, qt * 512 + off:(qt + 1) * 512],
                                start=True, stop=True)
                        pt2 = pt_pool.tile([128, 2, 512], MM_DT, tag="pt")
                        nc.scalar.activation(pt2[:], st2[:], Exp)
                        for j in range(2):
                            ki = ki2 + j
                            jd = ki - 4 * qt
                            if jd >= 0:       # diagonal block: causal mask
                                eng = (nc.vector if masks_state["flip"]
                                       else nc.gpsimd)
                                masks_state["flip"] ^= True
                                sl = slice(128 * jd, 128 * jd + 128)
                                eng.tensor_mul(pt2[:, j, sl], pt2[:, j, sl],
                                               tri[:])
                        for j in range(2):
                            ki = ki2 + j
                            jd = ki - 4 * qt
                            off = 128 * jd if jd > 0 else 0
                            _mm(nc, ot2[:, hh, off:512], v1[:, ki, h, :],
                                pt2[:, j, off:512],
                                start=(ki == 0), stop=(ki == nki - 1))
                # normalize the head pair. Evacuate PSUM first so ot2's two
                # banks free immediately (ot_ps is single-buffered).
                otsb = nrm_pool.tile([65, 2, 512], F32, tag="otsb", bufs=2)
                nc.vector.tensor_copy(otsb[:], ot2[:])
                recip = nrm_pool.tile([65, 2, 512], F32, tag="recip")
                nc.vector.reciprocal(recip[64:65, :, :], otsb[64:65, :, :])
                # partition_broadcast HW reads physical partition 0, so DMA
                # the row down to partition 0 first (DMA can shift partitions)
                stage = nrm_pool.tile([1, 2, 512], F32, tag="stage")
                nc.sync.dma_start(stage[:], recip[64:65, :, :])
                bcast = nrm_pool.tile([128, 2, 512], F32, tag="bcast")
                nc.gpsimd.partition_broadcast(
                    bcast[:].rearrange("p a b -> p (a b)"),
                    stage[:].rearrange("p a b -> p (a b)"))
                tsl = slice(qt * 512, (qt + 1) * 512)
                # even head -> yTp rows 0..63 (lane-aligned multiply)
                nc.vector.tensor_mul(yTp[0:64, hp, tsl],
                                     otsb[0:64, 0, :], bcast[0:64, 0, :])
                # odd head -> yTp rows 64..127: DVE cannot shift partitions,
                # so DMA the odd head's O^T down to partitions 64..127 first
                shifted = nrm_pool.tile([128, 512], F32, tag="shifted",
                                        bufs=2)
                nc.sync.dma_start(shifted[64:128, :], otsb[0:64, 1, :])
                nc.vector.tensor_mul(yTp[64:128, hp, tsl],
                                     shifted[64:128, :], bcast[64:128, 1, :])
            # out-projection for this qt's 4 token blocks
            for tb in range(qt * 4, (qt + 1) * 4):
                ops2 = op_ps.tile([128, 2, 512], F32, tag="ops2")
                for ct in range(2):
                    for hp2 in range(2):
                        _mm(nc, ops2[:, ct, :],
                            yTp[:, hp2, tb * 128:(tb + 1) * 128],
                            wo_sb[:, hp2, ct * 512:(ct + 1) * 512],
                            start=(hp2 == 0), stop=(hp2 == 1))
                if tb % 2 == 0:
                    osb2 = out_pool.tile([128, 2, C], F32, tag="osb2")
                nc.vector.tensor_copy(
                    osb2[:, tb % 2, :],
                    ops2[:].rearrange("p a b -> p (a b)"))
                if tb % 2 == 1:
                    nc.sync.dma_start(
                        yp[(tb - 1) * 128:(tb + 1) * 128, :]
                        .rearrange("(two p) c -> p two c", p=128),
                        osb2[:])


def build(repeat=1):
    nc = bacc.Bacc("TRN2", target_bir_lowering=False, debug=False,
                   enable_asserts=False, num_devices=NCORES)
    xb = nc.dram_tensor("xb", [T, C], F32, kind="ExternalInput").ap()
    wq = nc.dram_tensor("wq", [C, HCOLS], F32, kind="ExternalInput").ap()
    wk = nc.dram_tensor("wk", [C, HCOLS], F32, kind="ExternalInput").ap()
    wv = nc.dram_tensor("wv", [C, HCOLS], F32, kind="ExternalInput").ap()
    wo = nc.dram_tensor("wo", [HCOLS, C], F32, kind="ExternalInput").ap()
    yp = nc.dram_tensor("yp", [T, C], F32, kind="ExternalOutput").ap()

    masks_state = {"flip": False}
    with tile.TileContext(nc) as tc, ExitStack() as ctx:
        for _ in range(repeat):
            _build_rep(nc, tc, ctx, (xb, wq, wk, wv, wo, yp), masks_state)

    nc.compile()
    return nc


def make_in_maps(x, Wqkv, Wout):
    x = np.ascontiguousarray(np.asarray(x), dtype=np.float32)
    Wqkv = np.ascontiguousarray(np.asarray(Wqkv), dtype=np.float32)
    Wout = np.ascontiguousarray(np.asarray(Wout), dtype=np.float32)
    in_maps = []
    for c in range(NCORES):
        b, g = c // TPG, c % TPG
        lo, hi = g * HCOLS, (g + 1) * HCOLS
        in_maps.append({
            "xb": x[b],
            "wq": np.ascontiguousarray(Wqkv[:, lo:hi]),
            "wk": np.ascontiguousarray(Wqkv[:, C + lo:C + hi]),
            "wv": np.ascontiguousarray(Wqkv[:, 2 * C + lo:2 * C + hi]),
            "wo": np.ascontiguousarray(Wout[lo:hi, :]),
        })
    return in_maps


def combine_results(results):
    out = np.empty((B, T, C), dtype=np.float32)
    for b in range(B):
        out[b] = results[b * TPG]["yp"]
        for i in range(1, TPG):
            out[b] += results[b * TPG + i]["yp"]
    return out


def get_nc():
    if "nc" not in _CACHE:
        _CACHE["nc"] = build()
    return _CACHE["nc"]


def kernel(x, attn_mask, Wqkv, Wout):
    """Full inputs in, full output out. attn_mask is the causal tril mask
    (encoded in the kernel structure)."""
    x = np.asarray(x)
    assert x.shape == (B, T, C), x.shape
    assert np.asarray(Wqkv).shape == (C, 3 * C)
    assert np.asarray(Wout).shape == (C, C)
    nc = get_nc()
    in_maps = make_in_maps(x, Wqkv, Wout)
    res = run_bass_kernel_spmd(nc, in_maps, core_ids=list(range(NCORES)))
    return combine_results(res.results)


# revision 21
# speedup vs baseline: 1.0456x; 1.0456x over previous
"""Multi-head causal self-attention on 8 Trainium2 NeuronCores.

Problem: B=2, T=2048, C=1024, H=16 heads, D=64 head_dim, fp32.
  qkv = x @ Wqkv; causal softmax attention per head; y = attn_out @ Wout.

Sharding (2-way data parallel on batch x 4-way tensor parallel on heads):
  core c -> batch b = c // 4, head group g = c % 4 (heads 4g..4g+3).
  Each core computes its batch's QKV projection for its 4 heads, attention
  for those heads, and a partial out-projection (its 256 rows of Wout).
  The host sums the 4 partials per batch (the TP all-reduce).

Per-core kernel layout. All matmul operands live at SBUF partition base 0
(the PE rejects base-64 operands on this toolchain). "T"-suffixed tensors
are transposed, i.e. [feature, token] with feature on partitions:
  - x is PE-transposed once into xT [chan, tok]; 4 transposes share one
    PSUM bank and are evacuated with a single [128, 512] copy.
  - K^T head-pair blocks via matmul(lhsT=Wk_blk, rhs=xT) -> [col, tok].
  - Q^T per head in zero-padded [128, T] tiles qZ[h]: rows (h%2)*64..+64
    hold Q_h^T * 1/sqrt(D), the other 64 rows are zero, so a K=128
    contraction against the K^T pair block yields S^T for head h alone.
  - V in natural [tok, d] layout via matmul(lhsT=xT_blk, rhs=Wv), stored
    interleaved as [V_h | 1] per head (ones column -> denominator row).
  - S^T[k, q] = matmul(lhsT=K^T pair-block k-cols, rhs=qZ[h] q-tile), two
    k-blocks fused per [128, 1024] PSUM pair; one fused exp on ACT
    (no max subtraction: scores ~ N(0,1), fp32 exp is safe); diagonal
    blocks get a multiplicative 0/1 causal mask (split DVE / GpSimd).
  - O^T/denom = matmul(lhsT=[V_h|1], rhs=P^T) accumulated per head-pair in
    one [65, 2, 512] PSUM tile: rows 0..63 = O^T, row 64 = denominator.
  - normalize per head-pair: one reciprocal at partition 64 (lane-
    aligned), one SBUF->SBUF DMA down to partition 0 (DMA can shift
    partitions), one gpsimd partition_broadcast, one DVE mul -> yT.
  - partial out-proj: per token block accumulate 4 K=64 matmuls
    (lhsT=yT head slice, rhs=Wout rows) into a [128, 1024] PSUM pair;
    outputs are staged and DMA'd to HBM two token blocks at a time.

Matmuls run in fp32r (full PE rate; fp32 matmul is 4x slower). Set
MM_DT = mybir.dt.float32 below for the full-precision fallback.

build(repeat=N) unrolls the whole kernel N times back-to-back (used by
test.py to measure per-iteration device time without dispatch overhead).
"""

import math
from contextlib import ExitStack

import numpy as np

import concourse.bass as bass
import concourse.mybir as mybir
import concourse.tile as tile
from concourse import bacc
from concourse.bass_utils import run_bass_kernel_spmd
from concourse.masks import make_identity

B, T, C = 2, 2048, 1024
H, D = 16, 64
NCORES = 8
TPG = 4            # tensor-parallel group size (cores per batch)
HG = H // TPG      # heads per core = 4
HCOLS = HG * D     # 256 qkv columns per core
CB = C // 128      # 8 chan blocks
TB = T // 128      # 16 token blocks
QT = T // 512      # 4 q tiles
SCALE = 1.0 / math.sqrt(D)

F32 = mybir.dt.float32
MM_DT = mybir.dt.float32r   # matmul operand dtype (float32r = full PE rate)

_CACHE = {}


def _mm(nc, out, lhsT, rhs, **kw):
    nc.tensor.matmul(out, lhsT.bitcast(MM_DT), rhs.bitcast(MM_DT), **kw)


def _build_rep(nc, tc, ctx, tensors, masks_state):
    xb, wq, wk, wv, wo, yp = tensors
    Exp = mybir.ActivationFunctionType.Exp
    Copy = mybir.ActivationFunctionType.Copy

    with ExitStack() as pctx:
        # ---------- pools that live through the whole repetition ----------
        pp = pctx.enter_context(tc.tile_pool(name="pp", bufs=1))
        kT = pp.tile([128, 2 * T], MM_DT, tag="kT")
        qZ = [pp.tile([128, T], MM_DT, tag=f"qZ{h}", name=f"qZ{h}")
              for h in range(HG)]
        v1 = pp.tile([128, TB, HG, 65], MM_DT, tag="v1")

        with ExitStack() as phase12:
            sb1 = phase12.enter_context(tc.tile_pool(name="sb1", bufs=1))
            xT = sb1.tile([128, CB, T], MM_DT, tag="xT")       # 64KB/part
            wq_sb = sb1.tile([128, CB, HCOLS], MM_DT, tag="wq")
            wk_sb = sb1.tile([128, CB, HCOLS], MM_DT, tag="wk")
            wv_sb = sb1.tile([128, CB, HCOLS], MM_DT, tag="wv")
            ident = sb1.tile([128, 128], F32, tag="ident")
            make_identity(nc, ident[:])
            nc.sync.dma_start(wq_sb[:], wq.rearrange("(cb p) n -> p cb n", p=128).bitcast(MM_DT))
            nc.sync.dma_start(wk_sb[:], wk.rearrange("(cb p) n -> p cb n", p=128).bitcast(MM_DT))
            nc.sync.dma_start(wv_sb[:], wv.rearrange("(cb p) n -> p cb n", p=128).bitcast(MM_DT))
            # constants: ones columns of v1, zero halves of qZ
            nc.gpsimd.memset(v1[:, :, :, 64].bitcast(F32), 1.0)
            for h in range(HG):
                dead = slice(64, 128) if h % 2 == 0 else slice(0, 64)
                nc.gpsimd.memset(qZ[h][dead, :].bitcast(F32), 0.0)

            # PSUM budget 8 banks: tp(2) + vps(2) + qps(2) + kps(2).
            xn_pool = phase12.enter_context(tc.tile_pool(name="xn", bufs=2))
            tp_ps = phase12.enter_context(
                tc.tile_pool(name="tp_ps", bufs=2, space="PSUM"))
            v_ps = phase12.enter_context(
                tc.tile_pool(name="v_ps", bufs=2, space="PSUM"))
            qk_ps = phase12.enter_context(
                tc.tile_pool(name="qk_ps", bufs=2, space="PSUM"))

            # ---- phase 1: load x (2 tok blocks per DMA), transpose to xT,
            # ---- compute V per tok block
            for tb2 in range(TB // 2):
                xn = xn_pool.tile([128, 2, C], F32, tag="xn")
                nc.sync.dma_start(
                    xn[:], xb[tb2 * 256:(tb2 + 1) * 256, :]
                    .rearrange("(two p) c -> p two c", p=128))
                for half in range(2):
                    tb = 2 * tb2 + half
                    for cb4 in range(CB // 4):   # 4 transposes per PSUM bank
                        tp = tp_ps.tile([128, 512], F32, tag="tp")
                        for j in range(4):
                            cb = cb4 * 4 + j
                            nc.tensor.transpose(
                                tp[:, j * 128:(j + 1) * 128],
                                xn[:, half, cb * 128:(cb + 1) * 128],
                                ident[:])
                        # one evac for 4 transposed blocks: xT free-dim
                        # layout is [cb, tok], so this is a strided write
                        nc.vector.tensor_copy(
                            xT[:, cb4 * 4:(cb4 + 1) * 4,
                               tb * 128:(tb + 1) * 128],
                            tp[:].rearrange("p (cb t) -> p cb t", t=128))
                    vps = v_ps.tile([128, HCOLS], F32, tag="vps")
                    for cb in range(CB):
                        _mm(nc, vps[:], xT[:, cb, tb * 128:(tb + 1) * 128],
                            wv_sb[:, cb, :],
                            start=(cb == 0), stop=(cb == CB - 1))
                    nc.vector.tensor_copy(
                        v1[:, tb, :, 0:64],
                        vps[:].rearrange("p (h d) -> p h d", d=64))

            # ---- phase 2: K^T pair blocks + zero-padded scaled Q^T ----
            for m in range(2):
                for qt in range(QT):
                    qps = qk_ps.tile([128, 512], F32, tag="qps")
                    kps = qk_ps.tile([128, 512], F32, tag="kps")
                    for cb in range(CB):
                        _mm(nc, qps[:], wq_sb[:, cb, m * 128:(m + 1) * 128],
                            xT[:, cb, qt * 512:(qt + 1) * 512],
                            start=(cb == 0), stop=(cb == CB - 1))
                    for cb in range(CB):
                        _mm(nc, kps[:], wk_sb[:, cb, m * 128:(m + 1) * 128],
                            xT[:, cb, qt * 512:(qt + 1) * 512],
                            start=(cb == 0), stop=(cb == CB - 1))
                    tsl = slice(qt * 512, (qt + 1) * 512)
                    nc.scalar.activation(qZ[2 * m][0:64, tsl], qps[0:64, :],
                                         Copy, scale=SCALE)
                    nc.scalar.activation(qZ[2 * m + 1][64:128, tsl],
                                         qps[64:128, :], Copy, scale=SCALE)
                    nc.vector.tensor_copy(
                        kT[:, m * T + qt * 512: m * T + (qt + 1) * 512],
                        kps[:])

        # ---------- attention + out-projection ----------
        ap = pctx.enter_context(tc.tile_pool(name="ap", bufs=1))
        # Wout rows per head-PAIR at base 0: [128, pair, 1024] so the
        # out-projection contracts K=128 (heads 2hp and 2hp+1 together).
        wo_sb = ap.tile([128, 2, C], MM_DT, tag="wo")
        nc.sync.dma_start(wo_sb[:], wo.rearrange("(pb p) n -> p pb n", p=128).bitcast(MM_DT))
        # yTp[hp]: rows 0..63 = head 2hp, rows 64..127 = head 2hp+1
        yTp = ap.tile([128, 2, T], MM_DT, tag="yTp")
        # single triangular mask [128, 128]: 1 where qcol >= krow else 0.
        # On a diagonal k-block ki (j = ki - 4*qt), cols < 128j are fully
        # masked (skipped in the matmuls), cols in [128j, 128j+128) follow
        # this triangle, cols >= 128j+128 are fully valid.
        tri = ap.tile([128, 128], F32, tag="tri")
        nc.gpsimd.memset(tri[:], 1.0)
        nc.gpsimd.affine_select(
            out=tri[:], in_=tri[:], compare_op=mybir.AluOpType.is_ge,
            fill=0.0, base=0, pattern=[[1, 128]], channel_multiplier=-1)

        # Attention PSUM budget: st2(2x2) + ot2(1x2) + ops2(1x2) = 8 banks.
        st_ps = pctx.enter_context(tc.tile_pool(name="st_ps", bufs=2, space="PSUM"))
        ot_ps = pctx.enter_context(tc.tile_pool(name="ot_ps", bufs=1, space="PSUM"))
        op_ps = pctx.enter_context(tc.tile_pool(name="op_ps", bufs=1, space="PSUM"))
        pt_pool = pctx.enter_context(tc.tile_pool(name="pt", bufs=3))
        nrm_pool = pctx.enter_context(tc.tile_pool(name="nrm", bufs=1))
        out_pool = pctx.enter_context(tc.tile_pool(name="out", bufs=2))

        for qt in range(QT):
            nki = 4 * (qt + 1)
            for hp in range(2):          # head pairs (0,1) and (2,3)
                ot2 = ot_ps.tile([65, 2, 512], F32, tag="ot2")
                for ki2 in range(0, nki, 2):
                    for hh in range(2):
                        h = 2 * hp + hh
                        m = h // 2
                        st2 = st_ps.tile([128, 2, 512], F32, tag="st2")
                        for j in range(2):
                            ki = ki2 + j
                            jd = ki - 4 * qt
                            # skip fully-masked columns of diagonal blocks
                            # (keep qt==0 full so PSUM slots are always
                            # written over their full width at least once;
                            # stale cols feed exp and must stay finite)
                            off = 128 * jd if (jd > 0 and qt > 0) else 0
                            _mm(nc, st2[:, j, off:512],
                                kT[:, m * T + ki * 128: m * T + (ki + 1) * 128],
                                qZ[h][:, qt * 512 + off:(qt + 1) * 512],
                                start=True, stop=True)
                        pt2 = pt_pool.tile([128, 2, 512], MM_DT, tag="pt")
                        nc.scalar.activation(pt2[:], st2[:], Exp)
                        for j in range(2):
                            ki = ki2 + j
                            jd = ki - 4 * qt
                            if jd >= 0:       # diagonal block: causal mask
                                eng = (nc.vector if masks_state["flip"]
                                       else nc.gpsimd)
                                masks_state["flip"] ^= True
                                sl = slice(128 * jd, 128 * jd + 128)
                                eng.tensor_mul(pt2[:, j, sl], pt2[:, j, sl],
                                               tri[:])
                        for j in range(2):
                            ki = ki2 + j
                            jd = ki - 4 * qt
                            off = 128 * jd if jd > 0 else 0
                            _mm(nc, ot2[:, hh, off:512], v1[:, ki, h, :],
                                pt2[:, j, off:512],
                                start=(ki == 0), stop=(ki == nki - 1))
                # normalize the head pair. Evacuate PSUM first so ot2's two
                # banks free immediately (ot_ps is single-buffered).
                otsb = nrm_pool.tile([65, 2, 512], F32, tag="otsb", bufs=2)
                nc.vector.tensor_copy(otsb[:], ot2[:])
                recip = nrm_pool.tile([65, 2, 512], F32, tag="recip")
                nc.vector.reciprocal(recip[64:65, :, :], otsb[64:65, :, :])
                # partition_broadcast HW reads physical partition 0, so DMA
                # the row down to partition 0 first (DMA can shift partitions)
                stage = nrm_pool.tile([1, 2, 512], F32, tag="stage")
                nc.sync.dma_start(stage[:], recip[64:65, :, :])
                bcast = nrm_pool.tile([128, 2, 512], F32, tag="bcast")
                nc.gpsimd.partition_broadcast(
                    bcast[:].rearrange("p a b -> p (a b)"),
                    stage[:].rearrange("p a b -> p (a b)"))
                tsl = slice(qt * 512, (qt + 1) * 512)
                # even head -> yTp rows 0..63 (lane-aligned multiply)
                nc.vector.tensor_mul(yTp[0:64, hp, tsl],
                                     otsb[0:64, 0, :], bcast[0:64, 0, :])
                # odd head -> yTp rows 64..127: DVE cannot shift partitions,
                # so DMA the odd head's O^T down to partitions 64..127 first
                shifted = nrm_pool.tile([128, 512], F32, tag="shifted",
                                        bufs=2)
                nc.sync.dma_start(shifted[64:128, :], otsb[0:64, 1, :])
                nc.vector.tensor_mul(yTp[64:128, hp, tsl],
                                     shifted[64:128, :], bcast[64:128, 1, :])
            # out-projection for this qt's 4 token blocks
            for tb in range(qt * 4, (qt + 1) * 4):
                ops2 = op_ps.tile([128, 2, 512], F32, tag="ops2")
                for ct in range(2):
                    for hp2 in range(2):
                        _mm(nc, ops2[:, ct, :],
                            yTp[:, hp2, tb * 128:(tb + 1) * 128],
                            wo_sb[:, hp2, ct * 512:(ct + 1) * 512],
                            start=(hp2 == 0), stop=(hp2 == 1))
                if tb % 2 == 0:
                    osb2 = out_pool.tile([128, 2, C], F32, tag="osb2")
                nc.vector.tensor_copy(
                    osb2[:, tb % 2, :],
                    ops2[:].rearrange("p a b -> p (a b)"))
                if tb % 2 == 1:
                    nc.sync.dma_start(
                        yp[(tb - 1) * 128:(tb + 1) * 128, :]
                        .rearrange("(two p) c -> p two c", p=128),
                        osb2[:])


def build(repeat=1):
    nc = bacc.Bacc("TRN2", target_bir_lowering=False, debug=False,
                   enable_asserts=False, num_devices=NCORES)
    xb = nc.dram_tensor("xb", [T, C], F32, kind="ExternalInput").ap()
    wq = nc.dram_tensor("wq", [C, HCOLS], F32, kind="ExternalInput").ap()
    wk = nc.dram_tensor("wk", [C, HCOLS], F32, kind="ExternalInput").ap()
    wv = nc.dram_tensor("wv", [C, HCOLS], F32, kind="ExternalInput").ap()
    wo = nc.dram_tensor("wo", [HCOLS, C], F32, kind="ExternalInput").ap()
    yp = nc.dram_tensor("yp", [T, C], F32, kind="ExternalOutput").ap()

    masks_state = {"flip": False}
    with tile.TileContext(nc) as tc, ExitStack() as ctx:
        for _ in range(repeat):
            _build_rep(nc, tc, ctx, (xb, wq, wk, wv, wo, yp), masks_state)

    nc.compile()
    return nc


def make_in_maps(x, Wqkv, Wout):
    x = np.ascontiguousarray(np.asarray(x), dtype=np.float32)
    Wqkv = np.ascontiguousarray(np.asarray(Wqkv), dtype=np.float32)
    Wout = np.ascontiguousarray(np.asarray(Wout), dtype=np.float32)
    in_maps = []
    for c in range(NCORES):
        b, g = c // TPG, c % TPG
        lo, hi = g * HCOLS, (g + 1) * HCOLS
        in_maps.append({
            "xb": x[b],
            "wq": np.ascontiguousarray(Wqkv[:, lo:hi]),
            "wk": np.ascontiguousarray(Wqkv[:, C + lo:C + hi]),
            "wv": np.ascontiguousarray(Wqkv[:, 2 * C + lo:2 * C + hi]),
            "wo": np.ascontiguousarray(Wout[lo:hi, :]),
        })
    return in_maps


def combine_results(results):
    out = np.empty((B, T, C), dtype=np.float32)
    for b in range(B):
        out[b] = results[b * TPG]["yp"]
        for i in range(1, TPG):
            out[b] += results[b * TPG + i]["yp"]
    return out


def get_nc():
    if "nc" not in _CACHE:
        _CACHE["nc"] = build()
    return _CACHE["nc"]


def kernel(x, attn_mask, Wqkv, Wout):
    """Full inputs in, full output out. attn_mask is the causal tril mask
    (encoded in the kernel structure)."""
    x = np.asarray(x)
    assert x.shape == (B, T, C), x.shape
    assert np.asarray(Wqkv).shape == (C, 3 * C)
    assert np.asarray(Wout).shape == (C, C)
    nc = get_nc()
    in_maps = make_in_maps(x, Wqkv, Wout)
    res = run_bass_kernel_spmd(nc, in_maps, core_ids=list(range(NCORES)))
    return combine_results(res.results)
